# revision 16
# baseline (speedup 1.0000x reference)
"""MoE 2D router kernel for 8 Trainium2 NeuronCores — v4, transposed-space.

Strategy (pure data parallel, batch-sharded):
  - B=16 batches split across 8 cores (2 per core). Per core, each batch's
    [C=16, H=128, W=128] tensor is host-permuted to a [128, 2048] layout
    with partition p = c*8 + blk (blk = pixel-block of 2048 contiguous
    pixels); HBM loads are fully contiguous.
  - Everything is computed in TRANSPOSED (pixel-major) space: x and noise
    are PE-transposed (f32, bit-exact) per 128-column group, putting the
    expert axis on the free axis with stride 8. There:
      * top-1 / masked top-2 over experts are strided free-axis reduces,
      * per-pixel stats broadcast back over c as stride-0 views (no PE
        selection matmuls),
      * the softmax denominator is a free-axis add-reduce (no PE matmul),
      * G = mask * bcast(exp(m1)/ssum): the reciprocal is a tiny [128,64] op.
  - softplus(t) = Ln(1 + Exp(t)) on the combined exp/ln table; 1/wnoise =
    Exp(-Ln(wnoise)) on the same table; load = Erf(q) from the erf table.
    Erf runs in two groups with explicit ACT-queue ordering edges so the
    activation table is switched only 4 times total.
  - Work is processed as 4 virtual batches of [128, 1024]; elementwise work
    is split DVE/GPSIMD/ACT to balance engine busy times.
  - Outputs are written in transposed layout; the host inverts the
    permutation while unsharding.
"""
import sys

sys.path.insert(0, "/opt/trn_rl_repo")

import numpy as np

B, C, H, W = 16, 16, 128, 128
NCORES = 8
BPC = B // NCORES           # batches per core
HW = H * W                  # 16384 pixels per (batch, channel)
NBLK = 8                    # pixel blocks per batch (HW / 2048)
FB = C * HW // 128          # free size per batch in [128, FB] layout = 2048
VW = 1024                   # virtual-batch width
HPB = FB // VW              # halves per batch = 2
NVB = BPC * HPB             # virtual batches per core = 4
NG = VW // 128              # 128-col transpose groups per vbatch = 8

_CACHE = {}


def _build():
    import concourse.bacc as bacc
    import concourse.mybir as mybir
    from concourse.tile import TileContext, add_dep_helper

    f32 = mybir.dt.float32
    bf16 = mybir.dt.bfloat16
    AX = mybir.AxisListType
    OP = mybir.AluOpType
    AF = mybir.ActivationFunctionType
    BIGNEG = -1e30

    nc = bacc.Bacc(trn_type="TRN2", target_bir_lowering=False, debug=False,
                   num_devices=NCORES, name="moe_router")

    xd = nc.dram_tensor("x", [BPC, 128, FB], f32, kind="ExternalInput")
    nd = nc.dram_tensor("noise", [BPC, 128, FB], f32, kind="ExternalInput")
    idf_d = nc.dram_tensor("id_f", [128, 128], f32, kind="ExternalInput")
    wgpat_d = nc.dram_tensor("wg_pat", [128, VW], f32, kind="ExternalInput")
    wnpat_d = nc.dram_tensor("wn_pat", [128, VW], f32, kind="ExternalInput")
    gd = nc.dram_tensor("g_out", [BPC, HPB, 128, VW], f32,
                        kind="ExternalOutput")
    ld = nc.dram_tensor("load_out", [BPC, HPB, 128, VW], f32,
                        kind="ExternalOutput")

    with TileContext(nc) as tc:
        with tc.tile_pool(name="const", bufs=1) as cpool, \
             tc.tile_pool(name="io", bufs=2) as iop, \
             tc.tile_pool(name="work", bufs=2) as wp, \
             tc.tile_pool(name="small", bufs=2) as sp, \
             tc.tile_pool(name="erf", bufs=1) as ep, \
             tc.tile_pool(name="ps_t", bufs=2, space="PSUM") as ps_t:

            consts = [None]

            def _load_consts():
                idf = cpool.tile([128, 128], f32, tag="idf")
                nc.sync.dma_start(out=idf[:, :], in_=idf_d[:, :])
                wgpat = cpool.tile([128, VW], f32, tag="wgpat")
                nc.sync.dma_start(out=wgpat[:, :], in_=wgpat_d[:, :])
                wnpat = cpool.tile([128, VW], f32, tag="wnpat")
                nc.sync.dma_start(out=wnpat[:, :], in_=wnpat_d[:, :])
                return idf, wgpat, wnpat

            qts = []
            t6_by_half = [[], []]  # table-6 ACT instructions per kernel half

            def _emit_erf_group(group, after_insts):
                first = None
                prev = None
                for bb, hh, qt in group:
                    lt = iop.tile([128, VW], f32, tag=f"load{hh}")
                    i = nc.scalar.activation(lt[:, :], qt[:, :], AF.Erf)
                    if first is None:
                        first = i
                        for t6 in after_insts:
                            # add_dep_helper(a, b) == "a waits for b"
                            add_dep_helper(i.ins, t6.ins, sync=True,
                                           reason="erf after exp/ln group")
                    else:
                        add_dep_helper(i.ins, prev.ins, sync=True,
                                       reason="erf chain")
                    prev = i
                    nc.sync.dma_start(out=ld[bb, hh, :, :], in_=lt[:, :])
                return first, prev

            erf_a_last = [None]

            for vb in range(NVB):
                bb, hh = divmod(vb, HPB)
                hs = hh * VW
                half = vb // 2
                t6 = t6_by_half[half]

                # ---- load inputs ----
                xa = iop.tile([128, VW], f32, tag="x")
                nc.sync.dma_start(out=xa[:, :], in_=xd[bb, :, hs:hs + VW])
                na = iop.tile([128, VW], f32, tag="noise")
                nc.sync.dma_start(out=na[:, :], in_=nd[bb, :, hs:hs + VW])
                if consts[0] is None:
                    consts[0] = _load_consts()
                idf, wgpat, wnpat = consts[0]

                # ---- PE transposes to pixel-major (bit-exact f32) ----
                xT = ps_t.tile([128, VW], f32, tag="xT")
                nT = ps_t.tile([128, VW], f32, tag="nT")
                for g in range(NG):
                    s = slice(g * 128, (g + 1) * 128)
                    nc.tensor.transpose(xT[:, s], xa[:, s], idf[:, :])
                    nc.tensor.transpose(nT[:, s], na[:, s], idf[:, :])

                # ---- gates in T-space ----
                tv = wp.tile([128, VW], f32, tag="tv")
                nc.vector.tensor_tensor(tv[:, :], xT[:, :], wnpat[:, :],
                                        op=OP.mult)
                wg = wp.tile([128, VW], f32, tag="wg")
                nc.vector.tensor_tensor(wg[:, :], xT[:, :], wgpat[:, :],
                                        op=OP.mult)
                eu0 = wp.tile([128, VW], f32, tag="eu0")
                t6.append(nc.scalar.activation(eu0[:, :], tv[:, :], AF.Exp))
                wn = wp.tile([128, VW], f32, tag="wn")
                t6.append(nc.scalar.activation(wn[:, :], eu0[:, :], AF.Ln,
                                               bias=1.0))
                lw = wp.tile([128, VW], f32, tag="lw")
                t6.append(nc.scalar.activation(lw[:, :], wn[:, :], AF.Ln))
                rw = wp.tile([128, VW], f32, tag="rw")
                t6.append(nc.scalar.activation(rw[:, :], lw[:, :], AF.Exp,
                                               scale=-1.0))
                nw = wp.tile([128, VW], f32, tag="nw")
                nc.vector.tensor_tensor(nw[:, :], nT[:, :], wn[:, :],
                                        op=OP.mult)
                hl = wp.tile([128, VW], f32, tag="hl")
                nc.gpsimd.tensor_tensor(hl[:, :], wg[:, :], nw[:, :], op=OP.add)
                et = wp.tile([128, VW], bf16, tag="et")
                t6.append(nc.scalar.activation(et[:, :], hl[:, :], AF.Exp))

                # ---- expert-axis stats (contiguous free-axis reduces) ----
                vh = hl[:, :].rearrange("p (g c k) -> p g k c", g=NG, c=C)
                m1c = sp.tile([128, 64], f32, tag="m1c")
                nc.vector.tensor_reduce(m1c[:, :], vh, axis=AX.X, op=OP.max)
                m1b = (m1c[:, :].rearrange("p (g k) -> p g k", g=NG)
                       .unsqueeze(2).broadcast_to([128, NG, C, NBLK]))
                mk = wp.tile([128, VW], f32, tag="mk")
                nc.vector.tensor_tensor(mk[:, :], hl[:, :], m1b, op=OP.is_equal)
                md = wp.tile([128, VW], f32, tag="md")
                nc.vector.scalar_tensor_tensor(md[:, :], mk[:, :], BIGNEG,
                                               hl[:, :], op0=OP.mult, op1=OP.add)
                vm = md[:, :].rearrange("p (g c k) -> p g k c", g=NG, c=C)
                m2c = sp.tile([128, 64], f32, tag="m2c")
                nc.vector.tensor_reduce(m2c[:, :], vm, axis=AX.X, op=OP.max)
                s2c = sp.tile([128, 64], f32, tag="s2c")
                nc.vector.tensor_tensor(s2c[:, :], m2c[:, :], m1c[:, :],
                                        op=OP.subtract)
                ve = et[:, :].rearrange("p (g c k) -> p g k c", g=NG, c=C)
                ssc = sp.tile([128, 64], f32, tag="ssc")
                nc.vector.tensor_reduce(ssc[:, :], ve, axis=AX.X, op=OP.add)

                # ---- G = mask * bcast(exp(m1)/ssum) ----
                em = sp.tile([128, 64], f32, tag="em")
                t6.append(nc.scalar.activation(em[:, :], m1c[:, :], AF.Exp))
                src = sp.tile([128, 64], f32, tag="src")
                nc.vector.reciprocal(src[:, :], ssc[:, :])
                g1c = sp.tile([128, 64], f32, tag="g1c")
                nc.vector.tensor_tensor(g1c[:, :], em[:, :], src[:, :],
                                        op=OP.mult)
                g1b = (g1c[:, :].rearrange("p (g k) -> p g k", g=NG)
                       .unsqueeze(2).broadcast_to([128, NG, C, NBLK]))
                gt = iop.tile([128, VW], f32, tag="g")
                nc.gpsimd.tensor_tensor(gt[:, :], mk[:, :], g1b, op=OP.mult)
                nc.sync.dma_start(out=gd[bb, hh, :, :], in_=gt[:, :])

                # ---- erf argument: q = (wg - m1 - mk*(m2-m1)) / wnoise ----
                s2b = (s2c[:, :].rearrange("p (g k) -> p g k", g=NG)
                       .unsqueeze(2).broadcast_to([128, NG, C, NBLK]))
                d1 = wp.tile([128, VW], f32, tag="d1")
                nc.vector.tensor_tensor(d1[:, :], wg[:, :], m1b, op=OP.subtract)
                t1 = wp.tile([128, VW], f32, tag="t1")
                nc.gpsimd.tensor_tensor(t1[:, :], mk[:, :], s2b, op=OP.mult)
                numer = wp.tile([128, VW], f32, tag="numer")
                nc.gpsimd.tensor_tensor(numer[:, :], d1[:, :], t1[:, :],
                                        op=OP.subtract)
                qt = ep.tile([128, VW], f32, tag=f"q{vb}")
                nc.vector.tensor_tensor(qt[:, :], numer[:, :], rw[:, :],
                                        op=OP.mult)
                qts.append((bb, hh, qt))

            # ---- erf tail, strictly after every exp/ln ACT op ----
            _emit_erf_group(qts, t6_by_half[0] + t6_by_half[1])

    nc.compile()
    _fix_act_tables(nc, mybir)
    return nc


def _fix_act_tables(nc, mybir):
    """Retarget Exp/Ln activation-table loads to the combined exp+ln table
    and Erf loads to the erf-bearing table, then drop redundant reloads."""
    from concourse.hw_specs import get_activation_tables
    AFT = mybir.ActivationFunctionType
    tabs = list(get_activation_tables(nc.m.arch).items())
    targets = []
    for i, (_, fs) in enumerate(tabs):
        if AFT.Exp in fs and AFT.Ln in fs:
            targets.append((i, fs))
    for i, (_, fs) in enumerate(tabs):
        if AFT.Erf in fs:
            targets.append((i, fs))
    for blk in nc.m.functions[0].blocks:
        insts = blk.instructions
        loads = [(idx, inst) for idx, inst in enumerate(insts)
                 if isinstance(inst, mybir.InstLoadActFuncSet)]
        for li, (idx, load) in enumerate(loads):
            end = loads[li + 1][0] if li + 1 < len(loads) else len(insts)
            funcs = {i2.func for i2 in insts[idx + 1:end]
                     if isinstance(i2, mybir.InstActivation)}
            if not funcs:
                continue
            for tid, fs in targets:
                if funcs.issubset(fs):
                    load.act_func_set_id = tid
                    break
        cur = None
        to_remove = []
        for inst in insts:
            if isinstance(inst, mybir.InstLoadActFuncSet):
                if inst.act_func_set_id == cur and not inst.has_wait():
                    to_remove.append(inst)
                else:
                    cur = inst.act_func_set_id
            elif isinstance(inst, mybir.InstActivation):
                assert inst.func in tabs[cur][1], (inst.func, cur)
        for inst in to_remove:
            insts.remove(inst)


def make_in_maps(x, noise, wg_param, wnoise_param):
    identity = np.eye(128, dtype=np.float32)
    # partition layout p = c*8 + blk;  T-space free f = g*128 + c*8 + blk
    wgv = np.ascontiguousarray(wg_param, dtype=np.float32).reshape(C)
    wnv = np.ascontiguousarray(wnoise_param, dtype=np.float32).reshape(C)
    wg_pat = np.ascontiguousarray(
        np.broadcast_to(np.tile(np.repeat(wgv, NBLK), NG), (128, VW)))
    wn_pat = np.ascontiguousarray(
        np.broadcast_to(np.tile(np.repeat(wnv, NBLK), NG), (128, VW)))
    x = np.ascontiguousarray(x, dtype=np.float32)
    noise = np.ascontiguousarray(noise, dtype=np.float32)
    xp = x.reshape(B, 128, FB)
    npm = noise.reshape(B, 128, FB)
    in_maps = []
    for i in range(NCORES):
        in_maps.append({"x": xp[i * BPC:(i + 1) * BPC],
                        "noise": npm[i * BPC:(i + 1) * BPC],
                        "id_f": identity,
                        "wg_pat": wg_pat, "wn_pat": wn_pat})
    return in_maps


def _decode_T(arr):
    """[BPC, HPB, 128, VW] T-layout -> [BPC, C, H, W] standard layout.

    arr[bb, hh, pT, g*128 + c*8 + blk] = out[bb, c, blk*2048 + hh*1024
                                             + g*128 + pT]
    """
    a = np.asarray(arr, dtype=np.float32).reshape(BPC, HPB, 128, NG, C, NBLK)
    a = a.transpose(0, 4, 5, 1, 3, 2)  # [bb, c, blk, hh, g, pT]
    return a.reshape(BPC, C, H, W)


def kernel(x, noise, wg_param, wnoise_param):
    from concourse.bass_utils import run_bass_kernel_spmd

    if "nc" not in _CACHE:
        _CACHE["nc"] = _build()
    nc = _CACHE["nc"]
    in_maps = make_in_maps(x, noise, wg_param, wnoise_param)
    res = run_bass_kernel_spmd(nc, in_maps, list(range(NCORES)))
    G = np.empty((B, C, H, W), dtype=np.float32)
    L = np.empty((B, C, H, W), dtype=np.float32)
    for i in range(NCORES):
        G[i * BPC:(i + 1) * BPC] = _decode_T(res.results[i]["g_out"])
        L[i * BPC:(i + 1) * BPC] = _decode_T(res.results[i]["load_out"])
    return G, L


# revision 17
# speedup vs baseline: 1.2750x; 1.2750x over previous
"""MoE 2D router kernel for 8 Trainium2 NeuronCores — v5.

Strategy (pure data parallel, batch-sharded):
  - B=16 batches split across 8 cores (2 per core). Per core, each batch's
    [C=16, H=128, W=128] tensor is viewed as [128, 2048] with partition
    p = c*8 + blk (blk = pixel-block of 2048 contiguous pixels); HBM loads
    are fully contiguous.
  - Gates are computed in A-space where the channel params are
    per-partition scalars, so the parameter multiplies ride the ACT
    engine's scale input for free: wg = Copy(x; scale=wgp),
    wnoise = Ln(1 + Exp(x; scale=wnp)), nw = noise * wnoise.
  - hl = wg + nw is fused into the PE: two accumulating f32 transposes
    into one PSUM tile produce hl directly in pixel-major (T) space,
    bit-identical to an elementwise add. nw and wnoise are transposed
    alongside.
  - In T-space the expert axis c sits on the free axis (stride 8):
      * top-1 / masked top-2 over experts are strided free-axis reduces,
      * per-pixel stats broadcast back over c as stride-0 views,
      * the softmax denominator is a free-axis add-reduce,
      * G = mask * bcast(exp(m1)/ssum): reciprocal is a tiny [128,32] op,
      * the numerator uses u = hl - m1 (exact at the argmax):
        wg - max_excl = u - mask*(m2-m1) - nw.
  - softplus on the combined exp/ln table; 1/wnoise = Exp(-Ln(wnoise)) on
    the same table; load = Erf(q) from the erf table, executed in two
    groups with explicit ACT-queue ordering edges: 4 table loads total.
  - Outputs are written in transposed layout; the host inverts the
    permutation while unsharding.
"""
import sys

sys.path.insert(0, "/opt/trn_rl_repo")

import numpy as np

B, C, H, W = 16, 16, 128, 128
NCORES = 8
BPC = B // NCORES           # batches per core
HW = H * W                  # 16384 pixels per (batch, channel)
NBLK = 8                    # pixel blocks per batch (HW / 2048)
FB = C * HW // 128          # free size per batch in [128, FB] layout = 2048
CHW = 512                   # chunk width
NCH = CHW // 128            # 128-col transpose groups per chunk = 4
CPB = FB // CHW             # chunks per batch = 4
NCHUNK = BPC * CPB          # chunks per core = 8

_CACHE = {}


def _build():
    import concourse.bacc as bacc
    import concourse.mybir as mybir
    from concourse.tile import TileContext, add_dep_helper

    f32 = mybir.dt.float32
    bf16 = mybir.dt.bfloat16
    AX = mybir.AxisListType
    OP = mybir.AluOpType
    AF = mybir.ActivationFunctionType
    BIGNEG = -1e30

    nc = bacc.Bacc(trn_type="TRN2", target_bir_lowering=False, debug=False,
                   num_devices=NCORES, name="moe_router")

    xd = nc.dram_tensor("x", [BPC, 128, FB], f32, kind="ExternalInput")
    nd = nc.dram_tensor("noise", [BPC, 128, FB], f32, kind="ExternalInput")
    idf_d = nc.dram_tensor("id_f", [128, 128], f32, kind="ExternalInput")
    wgp_d = nc.dram_tensor("wgp", [128, 1], f32, kind="ExternalInput")
    wnp_d = nc.dram_tensor("wnp", [128, 1], f32, kind="ExternalInput")
    gd = nc.dram_tensor("g_out", [BPC, CPB, 128, CHW], f32,
                        kind="ExternalOutput")
    ld = nc.dram_tensor("load_out", [BPC, CPB, 128, CHW], f32,
                        kind="ExternalOutput")

    with TileContext(nc) as tc:
        with tc.tile_pool(name="const", bufs=1) as cpool, \
             tc.tile_pool(name="io", bufs=2) as iop, \
             tc.tile_pool(name="work", bufs=2) as wp, \
             tc.tile_pool(name="small", bufs=2) as sp, \
             tc.tile_pool(name="erf", bufs=1) as ep, \
             tc.tile_pool(name="ps_t", bufs=2, space="PSUM") as ps_t:

            consts = [None]

            def _load_consts():
                idf = cpool.tile([128, 128], f32, tag="idf")
                nc.sync.dma_start(out=idf[:, :], in_=idf_d[:, :])
                wgp = cpool.tile([128, 1], f32, tag="wgp")
                nc.sync.dma_start(out=wgp[:, :], in_=wgp_d[:, :])
                wnp = cpool.tile([128, 1], f32, tag="wnp")
                nc.sync.dma_start(out=wnp[:, :], in_=wnp_d[:, :])
                return idf, wgp, wnp

            qts = []
            t6_by_half = [[], []]  # table-6 ACT instructions per kernel half

            def _emit_erf_group(group, after_insts):
                first = None
                prev = None
                for bb, ch, qt in group:
                    lt = iop.tile([128, CHW], f32, tag=f"load{ch % 2}")
                    i = nc.scalar.activation(lt[:, :], qt[:, :], AF.Erf)
                    if first is None:
                        first = i
                        for t6 in after_insts:
                            # add_dep_helper(a, b) == "a waits for b"
                            add_dep_helper(i.ins, t6.ins, sync=True,
                                           reason="erf after exp/ln group")
                    else:
                        add_dep_helper(i.ins, prev.ins, sync=True,
                                       reason="erf chain")
                    prev = i
                    nc.sync.dma_start(out=ld[bb, ch, :, :], in_=lt[:, :])
                return first, prev

            erf_a_last = None

            for chunk in range(NCHUNK):
                bb, ch = divmod(chunk, CPB)
                cs = ch * CHW
                half = chunk // (NCHUNK // 2)
                t6 = t6_by_half[half]

                # ---- load inputs ----
                xa = iop.tile([128, CHW], f32, tag="x")
                nc.sync.dma_start(out=xa[:, :], in_=xd[bb, :, cs:cs + CHW])
                na = iop.tile([128, CHW], f32, tag="noise")
                nc.sync.dma_start(out=na[:, :], in_=nd[bb, :, cs:cs + CHW])
                if consts[0] is None:
                    consts[0] = _load_consts()
                idf, wgp, wnp = consts[0]

                # ---- gates in A-space (params are per-partition scalars) ----
                wga = wp.tile([128, CHW], f32, tag="wga")
                t6.append(nc.scalar.activation(wga[:, :], xa[:, :], AF.Copy,
                                               scale=wgp[:, :]))
                eu0 = wp.tile([128, CHW], f32, tag="eu0")
                t6.append(nc.scalar.activation(eu0[:, :], xa[:, :], AF.Exp,
                                               scale=wnp[:, :]))
                wn = wp.tile([128, CHW], f32, tag="wn")
                t6.append(nc.scalar.activation(wn[:, :], eu0[:, :], AF.Ln,
                                               bias=1.0))
                nwa = wp.tile([128, CHW], f32, tag="nwa")
                nc.gpsimd.tensor_tensor(nwa[:, :], na[:, :], wn[:, :],
                                        op=OP.mult)

                # ---- PE transposes; hl = T(wg) + T(nw) via PSUM accum ----
                hlT = ps_t.tile([128, CHW], f32, tag="hlT")
                nwT = ps_t.tile([128, CHW], f32, tag="nwT")
                wnT = ps_t.tile([128, CHW], f32, tag="wnT")
                for g in range(NCH):
                    s = slice(g * 128, (g + 1) * 128)
                    nc.tensor.matmul(hlT[:, s], wga[:, s], idf[:, :],
                                     is_transpose=True, start=True, stop=False)
                    nc.tensor.matmul(hlT[:, s], nwa[:, s], idf[:, :],
                                     is_transpose=True, start=False, stop=True)
                    nc.tensor.transpose(nwT[:, s], nwa[:, s], idf[:, :])
                    nc.tensor.transpose(wnT[:, s], wn[:, s], idf[:, :])

                # ---- T-space activations ----
                lwT = wp.tile([128, CHW], f32, tag="lwT")
                t6.append(nc.scalar.activation(lwT[:, :], wnT[:, :], AF.Ln))
                rwT = wp.tile([128, CHW], f32, tag="rwT")
                t6.append(nc.scalar.activation(rwT[:, :], lwT[:, :], AF.Exp,
                                               scale=-1.0))
                et = wp.tile([128, CHW], bf16, tag="et")
                t6.append(nc.scalar.activation(et[:, :], hlT[:, :], AF.Exp))

                # ---- expert-axis stats (strided free-axis reduces) ----
                vh = hlT[:, :].rearrange("p (g c k) -> p g k c", g=NCH, c=C)
                m1c = sp.tile([128, 32], f32, tag="m1c")
                nc.vector.tensor_reduce(m1c[:, :], vh, axis=AX.X, op=OP.max)
                m1b = (m1c[:, :].rearrange("p (g k) -> p g k", g=NCH)
                       .unsqueeze(2).broadcast_to([128, NCH, C, NBLK]))
                u = wp.tile([128, CHW], f32, tag="u")
                nc.vector.tensor_tensor(u[:, :], hlT[:, :], m1b, op=OP.subtract)
                mk = wp.tile([128, CHW], bf16, tag="mk")
                nc.vector.tensor_scalar(mk[:, :], u[:, :], 0.0, None,
                                        op0=OP.is_equal)
                md = wp.tile([128, CHW], f32, tag="md")
                nc.vector.scalar_tensor_tensor(md[:, :], mk[:, :], BIGNEG,
                                               u[:, :], op0=OP.mult, op1=OP.add)
                vm = md[:, :].rearrange("p (g c k) -> p g k c", g=NCH, c=C)
                s2c = sp.tile([128, 32], f32, tag="s2c")
                nc.vector.tensor_reduce(s2c[:, :], vm, axis=AX.X, op=OP.max)
                ve = et[:, :].rearrange("p (g c k) -> p g k c", g=NCH, c=C)
                ssc = sp.tile([128, 32], f32, tag="ssc")
                nc.vector.tensor_reduce(ssc[:, :], ve, axis=AX.X, op=OP.add)

                # ---- G = mask * bcast(exp(m1)/ssum) ----
                em = sp.tile([128, 32], f32, tag="em")
                t6.append(nc.scalar.activation(em[:, :], m1c[:, :], AF.Exp))
                src = sp.tile([128, 32], f32, tag="src")
                nc.vector.reciprocal(src[:, :], ssc[:, :])
                g1c = sp.tile([128, 32], f32, tag="g1c")
                nc.vector.tensor_tensor(g1c[:, :], em[:, :], src[:, :],
                                        op=OP.mult)
                g1b = (g1c[:, :].rearrange("p (g k) -> p g k", g=NCH)
                       .unsqueeze(2).broadcast_to([128, NCH, C, NBLK]))
                gt = iop.tile([128, CHW], f32, tag="g")
                nc.gpsimd.tensor_tensor(gt[:, :], mk[:, :], g1b, op=OP.mult)
                nc.sync.dma_start(out=gd[bb, ch, :, :], in_=gt[:, :])

                # ---- numer = u - mk*(m2-m1) - nw;  q = numer / wnoise ----
                s2b = (s2c[:, :].rearrange("p (g k) -> p g k", g=NCH)
                       .unsqueeze(2).broadcast_to([128, NCH, C, NBLK]))
                t1 = wp.tile([128, CHW], f32, tag="t1")
                nc.gpsimd.tensor_tensor(t1[:, :], mk[:, :], s2b, op=OP.mult)
                w1 = wp.tile([128, CHW], f32, tag="w1")
                nc.gpsimd.tensor_tensor(w1[:, :], u[:, :], t1[:, :],
                                        op=OP.subtract)
                numer = wp.tile([128, CHW], f32, tag="numer")
                nc.vector.tensor_tensor(numer[:, :], w1[:, :], nwT[:, :],
                                        op=OP.subtract)
                qt = ep.tile([128, CHW], f32, tag=f"q{chunk}")
                nc.vector.tensor_tensor(qt[:, :], numer[:, :], rwT[:, :],
                                        op=OP.mult)
                qts.append((bb, ch, qt))

                # ---- erf group A after first half, overlapping second ----
                if chunk == NCHUNK // 2 - 1:
                    _, erf_a_last = _emit_erf_group(qts[:NCHUNK // 2],
                                                    t6_by_half[0])

            # second half's table-6 ops wait for erf group A
            for t6 in t6_by_half[1]:
                add_dep_helper(t6.ins, erf_a_last.ins, sync=True,
                               reason="exp/ln group 2 after erf group A")
            _emit_erf_group(qts[NCHUNK // 2:], t6_by_half[1])

    nc.compile()
    _fix_act_tables(nc, mybir)
    return nc


def _fix_act_tables(nc, mybir):
    """Retarget Exp/Ln/Copy activation-table loads to the combined exp+ln
    table and Erf loads to the erf-bearing table, then drop redundant
    reloads."""
    from concourse.hw_specs import get_activation_tables
    AFT = mybir.ActivationFunctionType
    tabs = list(get_activation_tables(nc.m.arch).items())
    targets = []
    for i, (_, fs) in enumerate(tabs):
        if AFT.Exp in fs and AFT.Ln in fs:
            targets.append((i, fs))
    for i, (_, fs) in enumerate(tabs):
        if AFT.Erf in fs:
            targets.append((i, fs))
    for blk in nc.m.functions[0].blocks:
        insts = blk.instructions
        loads = [(idx, inst) for idx, inst in enumerate(insts)
                 if isinstance(inst, mybir.InstLoadActFuncSet)]
        for li, (idx, load) in enumerate(loads):
            end = loads[li + 1][0] if li + 1 < len(loads) else len(insts)
            funcs = {i2.func for i2 in insts[idx + 1:end]
                     if isinstance(i2, mybir.InstActivation)}
            if not funcs:
                continue
            for tid, fs in targets:
                if funcs.issubset(fs):
                    load.act_func_set_id = tid
                    break
        cur = None
        to_remove = []
        for inst in insts:
            if isinstance(inst, mybir.InstLoadActFuncSet):
                if inst.act_func_set_id == cur and not inst.has_wait():
                    to_remove.append(inst)
                else:
                    cur = inst.act_func_set_id
            elif isinstance(inst, mybir.InstActivation):
                assert inst.func in tabs[cur][1], (inst.func, cur)
        for inst in to_remove:
            insts.remove(inst)


def make_in_maps(x, noise, wg_param, wnoise_param):
    identity = np.eye(128, dtype=np.float32)
    wgv = np.ascontiguousarray(wg_param, dtype=np.float32).reshape(C)
    wnv = np.ascontiguousarray(wnoise_param, dtype=np.float32).reshape(C)
    # per-partition scalars for p = c*8 + blk
    wgp = np.repeat(wgv, NBLK).reshape(128, 1).astype(np.float32)
    wnp = np.repeat(wnv, NBLK).reshape(128, 1).astype(np.float32)
    x = np.ascontiguousarray(x, dtype=np.float32).reshape(B, 128, FB)
    noise = np.ascontiguousarray(noise, dtype=np.float32).reshape(B, 128, FB)
    in_maps = []
    for i in range(NCORES):
        in_maps.append({"x": x[i * BPC:(i + 1) * BPC],
                        "noise": noise[i * BPC:(i + 1) * BPC],
                        "id_f": identity, "wgp": wgp, "wnp": wnp})
    return in_maps


def _decode_T(arr):
    """[BPC, CPB, 128, CHW] T-layout -> [BPC, C, H, W] standard layout.

    arr[bb, ch, pT, g*128 + c*8 + blk] = out[bb, c, blk*2048 + ch*512
                                             + g*128 + pT]
    """
    a = np.asarray(arr, dtype=np.float32).reshape(BPC, CPB, 128, NCH, C, NBLK)
    a = a.transpose(0, 4, 5, 1, 3, 2)  # [bb, c, blk, ch, g, pT]
    return a.reshape(BPC, C, H, W)


def kernel(x, noise, wg_param, wnoise_param):
    from concourse.bass_utils import run_bass_kernel_spmd

    if "nc" not in _CACHE:
        _CACHE["nc"] = _build()
    nc = _CACHE["nc"]
    in_maps = make_in_maps(x, noise, wg_param, wnoise_param)
    res = run_bass_kernel_spmd(nc, in_maps, list(range(NCORES)))
    G = np.empty((B, C, H, W), dtype=np.float32)
    L = np.empty((B, C, H, W), dtype=np.float32)
    for i in range(NCORES):
        G[i * BPC:(i + 1) * BPC] = _decode_T(res.results[i]["g_out"])
        L[i * BPC:(i + 1) * BPC] = _decode_T(res.results[i]["load_out"])
    return G, L


# revision 18
# speedup vs baseline: 1.3567x; 1.0641x over previous
"""MoE 2D router kernel for 8 Trainium2 NeuronCores — v5.

Strategy (pure data parallel, batch-sharded):
  - B=16 batches split across 8 cores (2 per core). Per core, each batch's
    [C=16, H=128, W=128] tensor is viewed as [128, 2048] with partition
    p = c*8 + blk (blk = pixel-block of 2048 contiguous pixels); HBM loads
    are fully contiguous.
  - Gates are computed in A-space where the channel params are
    per-partition scalars, so the parameter multiplies ride the ACT
    engine's scale input for free: wg = Copy(x; scale=wgp),
    wnoise = Ln(1 + Exp(x; scale=wnp)), nw = noise * wnoise.
  - hl = wg + nw is fused into the PE: two accumulating f32 transposes
    into one PSUM tile produce hl directly in pixel-major (T) space,
    bit-identical to an elementwise add. nw and wnoise are transposed
    alongside.
  - In T-space the expert axis c sits on the free axis (stride 8):
      * top-1 / masked top-2 over experts are strided free-axis reduces,
      * per-pixel stats broadcast back over c as stride-0 views,
      * the softmax denominator is a free-axis add-reduce,
      * G = mask * bcast(exp(m1)/ssum): reciprocal is a tiny [128,32] op,
      * the numerator uses u = hl - m1 (exact at the argmax):
        wg - max_excl = u - mask*(m2-m1) - nw.
  - softplus on the combined exp/ln table; 1/wnoise = Exp(-Ln(wnoise)) on
    the same table; load = Erf(q) from the erf table, executed in two
    groups with explicit ACT-queue ordering edges: 4 table loads total.
  - Outputs are written in transposed layout; the host inverts the
    permutation while unsharding.
"""
import sys

sys.path.insert(0, "/opt/trn_rl_repo")

import numpy as np

B, C, H, W = 16, 16, 128, 128
NCORES = 8
BPC = B // NCORES           # batches per core
HW = H * W                  # 16384 pixels per (batch, channel)
NBLK = 8                    # pixel blocks per batch (HW / 2048)
FB = C * HW // 128          # free size per batch in [128, FB] layout = 2048
CHW = 512                   # chunk width
NCH = CHW // 128            # 128-col transpose groups per chunk = 4
CPB = FB // CHW             # chunks per batch = 4
NCHUNK = BPC * CPB          # chunks per core = 8

_CACHE = {}


def _build():
    import concourse.bacc as bacc
    import concourse.mybir as mybir
    from concourse.tile import TileContext, add_dep_helper

    f32 = mybir.dt.float32
    bf16 = mybir.dt.bfloat16
    AX = mybir.AxisListType
    OP = mybir.AluOpType
    AF = mybir.ActivationFunctionType
    BIGNEG = -1e30

    nc = bacc.Bacc(trn_type="TRN2", target_bir_lowering=False, debug=False,
                   num_devices=NCORES, name="moe_router")

    xd = nc.dram_tensor("x", [BPC, 128, FB], f32, kind="ExternalInput")
    nd = nc.dram_tensor("noise", [BPC, 128, FB], f32, kind="ExternalInput")
    idf_d = nc.dram_tensor("id_f", [128, 128], f32, kind="ExternalInput")
    wgp_d = nc.dram_tensor("wgp", [128, 1], f32, kind="ExternalInput")
    wnp_d = nc.dram_tensor("wnp", [128, 1], f32, kind="ExternalInput")
    gd = nc.dram_tensor("g_out", [BPC, CPB, 128, CHW], f32,
                        kind="ExternalOutput")
    ld = nc.dram_tensor("load_out", [BPC, CPB, 128, CHW], f32,
                        kind="ExternalOutput")

    with TileContext(nc) as tc:
        with tc.tile_pool(name="const", bufs=1) as cpool, \
             tc.tile_pool(name="io", bufs=2) as iop, \
             tc.tile_pool(name="work", bufs=2) as wp, \
             tc.tile_pool(name="small", bufs=2) as sp, \
             tc.tile_pool(name="erf", bufs=1) as ep, \
             tc.tile_pool(name="ps_t", bufs=2, space="PSUM") as ps_t:

            consts = [None]

            def _load_consts():
                idf = cpool.tile([128, 128], f32, tag="idf")
                nc.sync.dma_start(out=idf[:, :], in_=idf_d[:, :])
                wgp = cpool.tile([128, 1], f32, tag="wgp")
                nc.sync.dma_start(out=wgp[:, :], in_=wgp_d[:, :])
                wnp = cpool.tile([128, 1], f32, tag="wnp")
                nc.sync.dma_start(out=wnp[:, :], in_=wnp_d[:, :])
                return idf, wgp, wnp

            qts = []
            t6_by_half = [[], []]  # table-6 ACT instructions per kernel half

            def _emit_erf_group(group, after_insts):
                first = None
                prev = None
                for bb, ch, qt in group:
                    lt = iop.tile([128, CHW], f32, tag=f"load{ch % 2}")
                    i = nc.scalar.activation(lt[:, :], qt[:, :], AF.Erf)
                    if first is None:
                        first = i
                        for t6 in after_insts:
                            # add_dep_helper(a, b) == "a waits for b"
                            add_dep_helper(i.ins, t6.ins, sync=True,
                                           reason="erf after exp/ln group")
                    else:
                        add_dep_helper(i.ins, prev.ins, sync=True,
                                       reason="erf chain")
                    prev = i
                    nc.sync.dma_start(out=ld[bb, ch, :, :], in_=lt[:, :])
                return first, prev

            erf_a_last = None

            for chunk in range(NCHUNK):
                bb, ch = divmod(chunk, CPB)
                cs = ch * CHW
                half = chunk // (NCHUNK // 2)
                t6 = t6_by_half[half]

                # ---- load inputs ----
                xa = iop.tile([128, CHW], f32, tag="x")
                nc.sync.dma_start(out=xa[:, :], in_=xd[bb, :, cs:cs + CHW])
                na = iop.tile([128, CHW], f32, tag="noise")
                nc.sync.dma_start(out=na[:, :], in_=nd[bb, :, cs:cs + CHW])
                if consts[0] is None:
                    consts[0] = _load_consts()
                idf, wgp, wnp = consts[0]

                # ---- gates in A-space (params are per-partition scalars) ----
                wga = wp.tile([128, CHW], f32, tag="wga")
                t6.append(nc.scalar.activation(wga[:, :], xa[:, :], AF.Copy,
                                               scale=wgp[:, :]))
                eu0 = wp.tile([128, CHW], f32, tag="eu0")
                t6.append(nc.scalar.activation(eu0[:, :], xa[:, :], AF.Exp,
                                               scale=wnp[:, :]))
                wn = wp.tile([128, CHW], f32, tag="wn")
                t6.append(nc.scalar.activation(wn[:, :], eu0[:, :], AF.Ln,
                                               bias=1.0))
                nwa = wp.tile([128, CHW], f32, tag="nwa")
                nc.gpsimd.tensor_tensor(nwa[:, :], na[:, :], wn[:, :],
                                        op=OP.mult)

                # ---- PE transposes; hl = T(wg) + T(nw) via PSUM accum ----
                hlT = ps_t.tile([128, CHW], f32, tag="hlT")
                nwT = ps_t.tile([128, CHW], f32, tag="nwT")
                wnT = ps_t.tile([128, CHW], f32, tag="wnT")
                for g in range(NCH):
                    s = slice(g * 128, (g + 1) * 128)
                    nc.tensor.matmul(hlT[:, s], wga[:, s], idf[:, :],
                                     is_transpose=True, start=True, stop=False)
                    nc.tensor.matmul(hlT[:, s], nwa[:, s], idf[:, :],
                                     is_transpose=True, start=False, stop=True)
                    nc.tensor.transpose(nwT[:, s], nwa[:, s], idf[:, :])
                    nc.tensor.transpose(wnT[:, s], wn[:, s], idf[:, :])

                # ---- T-space activations ----
                lwT = wp.tile([128, CHW], f32, tag="lwT")
                t6.append(nc.scalar.activation(lwT[:, :], wnT[:, :], AF.Ln))
                rwT = wp.tile([128, CHW], f32, tag="rwT")
                t6.append(nc.scalar.activation(rwT[:, :], lwT[:, :], AF.Exp,
                                               scale=-1.0))
                et = wp.tile([128, CHW], bf16, tag="et")
                t6.append(nc.scalar.activation(et[:, :], hlT[:, :], AF.Exp))

                # ---- expert-axis stats (strided free-axis reduces) ----
                vh = hlT[:, :].rearrange("p (g c k) -> p g k c", g=NCH, c=C)
                m1c = sp.tile([128, 32], f32, tag="m1c")
                nc.vector.tensor_reduce(m1c[:, :], vh, axis=AX.X, op=OP.max)
                m1b = (m1c[:, :].rearrange("p (g k) -> p g k", g=NCH)
                       .unsqueeze(2).broadcast_to([128, NCH, C, NBLK]))
                u = wp.tile([128, CHW], f32, tag="u")
                nc.vector.tensor_tensor(u[:, :], hlT[:, :], m1b, op=OP.subtract)
                mk = wp.tile([128, CHW], bf16, tag="mk")
                nc.vector.tensor_scalar(mk[:, :], u[:, :], 0.0, None,
                                        op0=OP.is_equal)
                md = wp.tile([128, CHW], f32, tag="md")
                nc.vector.scalar_tensor_tensor(md[:, :], mk[:, :], BIGNEG,
                                               u[:, :], op0=OP.mult, op1=OP.add)
                vm = md[:, :].rearrange("p (g c k) -> p g k c", g=NCH, c=C)
                s2c = sp.tile([128, 32], f32, tag="s2c")
                nc.vector.tensor_reduce(s2c[:, :], vm, axis=AX.X, op=OP.max)
                ve = et[:, :].rearrange("p (g c k) -> p g k c", g=NCH, c=C)
                ssc = sp.tile([128, 32], f32, tag="ssc")
                nc.vector.tensor_reduce(ssc[:, :], ve, axis=AX.X, op=OP.add)

                # ---- G = mask * bcast(exp(m1)/ssum) ----
                em = sp.tile([128, 32], f32, tag="em")
                t6.append(nc.scalar.activation(em[:, :], m1c[:, :], AF.Exp))
                src = sp.tile([128, 32], f32, tag="src")
                nc.vector.reciprocal(src[:, :], ssc[:, :])
                g1c = sp.tile([128, 32], f32, tag="g1c")
                nc.vector.tensor_tensor(g1c[:, :], em[:, :], src[:, :],
                                        op=OP.mult)
                g1b = (g1c[:, :].rearrange("p (g k) -> p g k", g=NCH)
                       .unsqueeze(2).broadcast_to([128, NCH, C, NBLK]))
                gt = iop.tile([128, CHW], f32, tag="g")
                nc.gpsimd.tensor_tensor(gt[:, :], mk[:, :], g1b, op=OP.mult)
                nc.sync.dma_start(out=gd[bb, ch, :, :], in_=gt[:, :])

                # ---- numer = u - mk*(m2-m1) - nw;  q = numer / wnoise ----
                s2b = (s2c[:, :].rearrange("p (g k) -> p g k", g=NCH)
                       .unsqueeze(2).broadcast_to([128, NCH, C, NBLK]))
                t1 = wp.tile([128, CHW], f32, tag="t1")
                nc.gpsimd.tensor_tensor(t1[:, :], mk[:, :], s2b, op=OP.mult)
                w1 = wp.tile([128, CHW], f32, tag="w1")
                nc.gpsimd.tensor_tensor(w1[:, :], u[:, :], t1[:, :],
                                        op=OP.subtract)
                numer = wp.tile([128, CHW], f32, tag="numer")
                nc.vector.tensor_tensor(numer[:, :], w1[:, :], nwT[:, :],
                                        op=OP.subtract)
                qt = ep.tile([128, CHW], f32, tag=f"q{chunk}")
                nc.vector.tensor_tensor(qt[:, :], numer[:, :], rwT[:, :],
                                        op=OP.mult)
                qts.append((bb, ch, qt))

                # ---- erf group A after first half, overlapping second ----
                if chunk == NCHUNK // 2 - 1:
                    _, erf_a_last = _emit_erf_group(qts[:NCHUNK // 2],
                                                    t6_by_half[0])

            _emit_erf_group(qts[NCHUNK // 2:], t6_by_half[1])

    nc.compile()
    _fix_act_tables(nc, mybir)
    return nc


def _fix_act_tables(nc, mybir):
    """Retarget Exp/Ln/Copy activation-table loads to the combined exp+ln
    table and Erf loads to the erf-bearing table, then drop redundant
    reloads."""
    from concourse.hw_specs import get_activation_tables
    AFT = mybir.ActivationFunctionType
    tabs = list(get_activation_tables(nc.m.arch).items())
    targets = []
    for i, (_, fs) in enumerate(tabs):
        if AFT.Exp in fs and AFT.Ln in fs:
            targets.append((i, fs))
    for i, (_, fs) in enumerate(tabs):
        if AFT.Erf in fs:
            targets.append((i, fs))
    for blk in nc.m.functions[0].blocks:
        insts = blk.instructions
        loads = [(idx, inst) for idx, inst in enumerate(insts)
                 if isinstance(inst, mybir.InstLoadActFuncSet)]
        for li, (idx, load) in enumerate(loads):
            end = loads[li + 1][0] if li + 1 < len(loads) else len(insts)
            funcs = {i2.func for i2 in insts[idx + 1:end]
                     if isinstance(i2, mybir.InstActivation)}
            if not funcs:
                continue
            for tid, fs in targets:
                if funcs.issubset(fs):
                    load.act_func_set_id = tid
                    break
        cur = None
        to_remove = []
        for inst in insts:
            if isinstance(inst, mybir.InstLoadActFuncSet):
                if inst.act_func_set_id == cur and not inst.has_wait():
                    to_remove.append(inst)
                else:
                    cur = inst.act_func_set_id
            elif isinstance(inst, mybir.InstActivation):
                assert inst.func in tabs[cur][1], (inst.func, cur)
        for inst in to_remove:
            insts.remove(inst)


def make_in_maps(x, noise, wg_param, wnoise_param):
    identity = np.eye(128, dtype=np.float32)
    wgv = np.ascontiguousarray(wg_param, dtype=np.float32).reshape(C)
    wnv = np.ascontiguousarray(wnoise_param, dtype=np.float32).reshape(C)
    # per-partition scalars for p = c*8 + blk
    wgp = np.repeat(wgv, NBLK).reshape(128, 1).astype(np.float32)
    wnp = np.repeat(wnv, NBLK).reshape(128, 1).astype(np.float32)
    x = np.ascontiguousarray(x, dtype=np.float32).reshape(B, 128, FB)
    noise = np.ascontiguousarray(noise, dtype=np.float32).reshape(B, 128, FB)
    in_maps = []
    for i in range(NCORES):
        in_maps.append({"x": x[i * BPC:(i + 1) * BPC],
                        "noise": noise[i * BPC:(i + 1) * BPC],
                        "id_f": identity, "wgp": wgp, "wnp": wnp})
    return in_maps


def _decode_T(arr):
    """[BPC, CPB, 128, CHW] T-layout -> [BPC, C, H, W] standard layout.

    arr[bb, ch, pT, g*128 + c*8 + blk] = out[bb, c, blk*2048 + ch*512
                                             + g*128 + pT]
    """
    a = np.asarray(arr, dtype=np.float32).reshape(BPC, CPB, 128, NCH, C, NBLK)
    a = a.transpose(0, 4, 5, 1, 3, 2)  # [bb, c, blk, ch, g, pT]
    return a.reshape(BPC, C, H, W)


def kernel(x, noise, wg_param, wnoise_param):
    from concourse.bass_utils import run_bass_kernel_spmd

    if "nc" not in _CACHE:
        _CACHE["nc"] = _build()
    nc = _CACHE["nc"]
    in_maps = make_in_maps(x, noise, wg_param, wnoise_param)
    res = run_bass_kernel_spmd(nc, in_maps, list(range(NCORES)))
    G = np.empty((B, C, H, W), dtype=np.float32)
    L = np.empty((B, C, H, W), dtype=np.float32)
    for i in range(NCORES):
        G[i * BPC:(i + 1) * BPC] = _decode_T(res.results[i]["g_out"])
        L[i * BPC:(i + 1) * BPC] = _decode_T(res.results[i]["load_out"])
    return G, L
